# revision 15
# baseline (speedup 1.0000x reference)
"""Depthwise 4D conv (3,3,3,3) kernel for Trainium2, 8 NeuronCores.

Problem: inputs [4, 64, 32, 32, 32, 8] f32, kernel [81, 64, 1] f32 (per-tap,
per-channel scalar weights), 'same' padding, stride 1 -> output same shape.

Strategy
--------
Sharding: channel-parallel. 64 channels / 8 cores = 8 channels per core;
each core handles all 4 batches for its channels (weights are per-channel,
so they shard cleanly with the data and every core's work is identical).

Compute mapping (per core): per-channel 2D block-Toeplitz matmul on the PE.
For one (channel c, batch b, x-row slot s covering output rows {2s, 2s+1}):

  out[xr*32+yo, (s, z, t)] = sum_K  W[(dx, yin), (xr, yo)] * X[s][(dx,yin), zt]

with contraction K = 128 = 4 padded-x-rows (dx) x 32 y-rows (yin) and output
M = 64 = 2 x-rows x 32 y-rows.  The Toeplitz weight block W encodes the
(k1, k2) taps (y-edges are structural zeros in the block); the (k3, k4)
taps are 9 accumulating matmul passes reading the SAME rhs tile through a
shifted 2D (z, t) access pattern, into one PSUM tile.

Key exploits vs the naive mapping:
 - The Toeplitz block is shift-invariant in x, so the 4 slots of an 8-row
   x-group are a FREE dimension of a single matmul (slot-major rhs AP),
   not separate matmuls: 2 slots x 32 z x 8 t = 512 columns per matmul.
   This quarters the instruction count and the LDWEIGHTS traffic.
 - The rhs AP is (slot, z, t) strided and each tap pass streams only its
   VALID output window (center-first pass order: the (1,1) tap covers the
   full range with start=True, edge taps accumulate onto sub-windows), so
   no z/t padding exists anywhere -- neither in DRAM nor in the streamed
   columns (2068 columns per slot per 9 passes instead of 9*320).
 - Two x-groups (ggg = 2*pair, 2*pair+1) run CONCURRENTLY on the two
   column halves of the PE array via tile_position column tiling.
 - Operands are bf16 (fp32 PSUM accumulation); output is written bf16
   (rel-err budget 2e-2, bf16 rounding adds ~2e-3).
 - Output DMAs issue from the scalar engine's HWDGE queue so their
   extraction-semaphore waits can't head-of-line block the input
   prefetch DMAs on the sync queue (that stall was worth ~25%).

Boundary handling:
 - y edges: structural zeros in the Toeplitz weight blocks,
 - x edges: host pads x with one zero row on each side (34 rows),
 - z/t edges: valid-window matmul passes (see above).
"""

import os
import sys

import numpy as np

for _p in ("/opt/trn_rl_repo",):
    if _p not in sys.path and os.path.isdir(_p):
        sys.path.insert(0, _p)

B, C, X, Y, Z, T = 4, 64, 32, 32, 32, 8
N_CORES = 8
CH_PER_CORE = C // N_CORES
XP = X + 2                                 # x padded extent (34)
NFREE = Z * T                              # 256: per-(x,y) zt-plane width

# (dz, dt) tap pass order: center tap first -- it covers the FULL (z, t)
# output range, so its start=True resets every PSUM address; the 8 edge
# taps then accumulate onto valid sub-windows (shorter N, no padding).
PASS_ORDER = [(1, 1)] + [(dz, dt) for dz in range(3) for dt in range(3)
                         if (dz, dt) != (1, 1)]

LAST_EXEC_NS = None


def _build_lhsT_all(kernel_np: np.ndarray) -> np.ndarray:
    """kernel [81, C, 1] -> lhsT blocks [C, 9, 128, 64].

    lhsT[c, p, dx*32 + yin, xr*32 + yo] = w4[k1, k2, k3, k4, c]
    where p = k3*3 + k4, dx = xr + k1, yin = yo + k2 - 1 (only where
    0 <= yin < 32 -- y-boundary zeros live in the matrix).
    """
    w4 = kernel_np.reshape(3, 3, 3, 3, C).astype(np.float32)
    lhsT = np.zeros((C, 9, 128, 64), np.float32)
    yo = np.arange(Y)
    for k1 in range(3):
        for xr in range(2):
            dx = xr + k1
            for k2 in range(3):
                yi = yo + k2 - 1
                m = (yi >= 0) & (yi < Y)
                rows = dx * 32 + yi[m]
                cols = xr * 32 + yo[m]
                for k3 in range(3):
                    for k4 in range(3):
                        p = k3 * 3 + k4
                        lhsT[:, p, rows, cols] = w4[k1, k2, k3, k4][:, None]
    return lhsT


def _pad_core_input(x_core: np.ndarray) -> np.ndarray:
    """[B, ch, X, Y, Z, T] -> flat bf16 x-padded [B*ch*XP*Y*Z*T].

    Only x is zero-padded (34 rows); z/t boundary taps are handled on
    device by shortened valid-window matmul passes, so no z/t padding.
    """
    import ml_dtypes
    bf16 = ml_dtypes.bfloat16
    arr = np.zeros((B, CH_PER_CORE, XP, Y, Z, T), bf16)
    arr[:, :, 1 : X + 1] = x_core.astype(bf16)
    return arr.ravel()


_NC_CACHE: dict = {}


def _get_nc(repeats: int = 1):
    key = ("nc", repeats)
    if key in _NC_CACHE:
        return _NC_CACHE[key]

    import concourse.mybir as mybir
    from concourse import bacc
    from concourse.bass import AP
    from concourse.tile import TileContext

    f32 = mybir.dt.float32
    bf16 = mybir.dt.bfloat16
    nc = bacc.Bacc("TRN2", target_bir_lowering=False, debug=False,
                   num_devices=N_CORES)

    n_xpad = B * CH_PER_CORE * XP * Y * NFREE
    xpad = nc.dram_tensor("xpad", (n_xpad,), bf16,
                          kind="ExternalInput").ap()
    wts = nc.dram_tensor("wts", (CH_PER_CORE, 128, 9 * 64), bf16,
                         kind="ExternalInput").ap()
    out = nc.dram_tensor("out", (B, CH_PER_CORE, X, Y, Z, T), bf16,
                         kind="ExternalOutput").ap()

    # strides (elements) within xpad for manual AP construction
    s_y = NFREE              # 256
    s_x = Y * NFREE          # 8192
    # strides (elements) within out per (b, ci)
    o_x = Y * Z * T          # 8192
    o_blk = X * o_x          # per (b, ci) block

    with TileContext(nc) as tc:
        with tc.tile_pool(name="w", bufs=2) as wpool, \
             tc.tile_pool(name="in", bufs=14) as ipool, \
             tc.tile_pool(name="out", bufs=8) as opool, \
             tc.tile_pool(name="ps", bufs=8, space="PSUM") as pspool:
            for ci in [c for _ in range(repeats) for c in range(CH_PER_CORE)]:
                wtile = wpool.tile([128, 9 * 64], bf16, tag="w")
                nc.sync.dma_start(out=wtile[:], in_=wts[ci])
                for b in range(B):
                    for pair in range(2):
                        # Two itiles (one per PE column group): group g
                        # covers x-group ggg = 2*pair + g, i.e. padded
                        # x-rows 8*ggg .. 8*ggg+9 as 4 stride-2 slots of
                        # 4 rows x 32 y on the partitions.
                        itiles = []
                        for g in range(2):
                            it = ipool.tile([128, 4 * NFREE], bf16,
                                            tag="in", name=f"it{g}")
                            base_off = ((b * CH_PER_CORE + ci) * XP
                                        + 16 * pair + 8 * g) * s_x
                            src = AP(xpad.tensor, base_off,
                                     [[s_x, 4], [s_y, Y], [2 * s_x, 4],
                                      [1, NFREE]])
                            dv = it[:].rearrange("p (s w) -> p s w",
                                                 w=NFREE)
                            nc.sync.dma_start(out=dv, in_=src)
                            itiles.append(it)

                        # (slot, z, t) views for shifted rhs APs
                        views = [it[:].rearrange(
                            "p (s z t) -> p s z t", z=Z, t=T)
                            for it in itiles]

                        pts = [pspool.tile([128, 2 * Z * T], f32, tag="ps",
                                           name=f"pt{h}")
                               for h in range(2)]
                        ptvs = [pt[:].rearrange("p (s z t) -> p s z t",
                                                z=Z, t=T)
                                for pt in pts]
                        for pi, (dz, dt) in enumerate(PASS_ORDER):
                            p9 = dz * 3 + dt
                            # valid output window for this tap: out (z,t)
                            # reads input (z+dz-1, t+dt-1)
                            z0, z1 = max(0, 1 - dz), Z - max(0, dz - 1)
                            t0, t1 = max(0, 1 - dt), T - max(0, dt - 1)
                            zi, ti = z0 + dz - 1, t0 + dt - 1
                            for h in range(2):
                                for g in range(2):
                                    rhs = views[g][:, 2 * h : 2 * h + 2,
                                                   zi : zi + (z1 - z0),
                                                   ti : ti + (t1 - t0)]
                                    dst = ptvs[h][
                                        64 * g : 64 * (g + 1),
                                        :, z0:z1, t0:t1]
                                    nc.tensor.matmul(
                                        dst,
                                        lhsT=wtile[:,
                                                   p9 * 64 : (p9 + 1) * 64],
                                        rhs=rhs,
                                        start=(pi == 0),
                                        stop=(pi == 8),
                                        tile_position=(0, 64 * g),
                                    )

                        # PSUM -> SBUF (f32 -> bf16), split across two
                        # engines so neither serializes the PE.
                        otile = opool.tile([128, 4 * Z * T], bf16,
                                           tag="out")
                        nc.vector.tensor_copy(
                            out=otile[:, 0 : 2 * Z * T], in_=pts[0][:])
                        nc.scalar.activation(
                            out=otile[:, 2 * Z * T : 4 * Z * T],
                            in_=pts[1][:],
                            func=mybir.ActivationFunctionType.Identity)

                        # otile partition p = 64g + 32xr + yo,
                        # free = (s in 4, zt in 256);
                        # out x-row = 16*pair + 8g + 2s + xr.
                        # One DMA per g so each side's AP merges to <=3
                        # dims (the x footprint interleaves partition xr
                        # with free s, so a single 5-dim dst won't).
                        for g in range(2):
                            obase = ((b * CH_PER_CORE + ci) * o_blk
                                     + (16 * pair + 8 * g) * o_x)
                            dst = AP(out.tensor, obase,
                                     [[o_x, 2], [Z * T, Y],
                                      [2 * o_x, 4], [1, Z * T]])
                            # Issue output DMAs from the scalar engine's
                            # HWDGE queue: on the sync queue their
                            # extraction-semaphore waits head-of-line
                            # block the input prefetch DMAs behind them
                            # and starve the PE.
                            nc.scalar.dma_start(
                                out=dst,
                                in_=otile[64 * g : 64 * (g + 1)].rearrange(
                                    "p (s zt) -> p s zt", zt=Z * T))

    nc.finalize()
    _NC_CACHE[key] = nc
    return nc


def _get_runner():
    """Build (once) a cached jitted SPMD executable for the Bass program.

    Mirrors bass2jax.run_bass_via_pjrt's multi-core path, but without
    output-buffer donation (the kernel writes every output element) so the
    compiled callable can be invoked repeatedly with device-resident args
    for steady-state timing.
    """
    return _get_runner_r(1)


def _get_runner_r(repeats: int):
    key = ("runner", repeats)
    if key in _NC_CACHE:
        return _NC_CACHE[key]

    import jax
    import concourse.mybir as mybir
    from concourse import bass2jax
    from concourse.bass2jax import _bass_exec_p, install_neuronx_cc_hook
    from jax.experimental.shard_map import shard_map
    from jax.sharding import Mesh, NamedSharding, PartitionSpec

    nc = _get_nc(repeats)
    install_neuronx_cc_hook()

    partition_name = (
        nc.partition_id_tensor.name if nc.partition_id_tensor else None
    )
    in_names, out_names, out_avals, zero_outs = [], [], [], []
    for alloc in nc.m.functions[0].allocations:
        if not isinstance(alloc, mybir.MemoryLocationSet):
            continue
        name = alloc.memorylocations[0].name
        if alloc.kind == "ExternalInput":
            if name != partition_name:
                in_names.append(name)
        elif alloc.kind == "ExternalOutput":
            shape = tuple(alloc.tensor_shape)
            dtype = mybir.dt.np(alloc.dtype)
            out_names.append(name)
            out_avals.append(jax.core.ShapedArray(shape, dtype))
            zero_outs.append(np.zeros(shape, dtype))
    n_params = len(in_names)
    all_in_names = list(in_names) + list(out_names)
    if partition_name is not None:
        all_in_names.append(partition_name)

    def _body(*args):
        operands = list(args)
        if partition_name is not None:
            operands.append(bass2jax.partition_id_tensor())
        outs = _bass_exec_p.bind(
            *operands,
            out_avals=tuple(out_avals),
            in_names=tuple(all_in_names),
            out_names=tuple(out_names),
            lowering_input_output_aliases=(),
            sim_require_finite=True,
            sim_require_nnan=True,
            nc=nc,
        )
        return tuple(outs)

    devices = jax.devices()[:N_CORES]
    mesh = Mesh(np.asarray(devices), ("core",))
    spec = PartitionSpec("core")
    n_args = n_params + len(out_names)
    sharded = jax.jit(
        shard_map(_body, mesh=mesh, in_specs=(spec,) * n_args,
                  out_specs=(spec,) * len(out_names), check_rep=False),
        keep_unused=True,
    )
    sharding = NamedSharding(mesh, spec)

    def run(in_maps, timing_reps=0, profile_hook=None):
        concat_in = [
            np.concatenate([np.asarray(in_maps[c][name])
                            for c in range(N_CORES)], axis=0)
            for name in in_names
        ]
        concat_zero = [
            np.zeros((N_CORES * z.shape[0], *z.shape[1:]), z.dtype)
            for z in zero_outs
        ]
        dev_args = [jax.device_put(a, sharding)
                    for a in (*concat_in, *concat_zero)]
        out_arrs = jax.block_until_ready(sharded(*dev_args))

        exec_ns = None
        if timing_reps > 0:
            import time
            sharded(*dev_args)  # extra warmup
            jax.block_until_ready(sharded(*dev_args))
            t0 = time.perf_counter()
            for _ in range(timing_reps):
                last = sharded(*dev_args)
            jax.block_until_ready(last)
            exec_ns = (time.perf_counter() - t0) / timing_reps * 1e9

        if profile_hook is not None:
            # Extra (already-warm) executions under the caller's
            # profiler context(s) (e.g. the axon NTFF capture hook).
            # A list profiles one execution per hook so the caller can
            # take the best complete execution (run-to-run DMA jitter
            # is ~2% on the max core).
            hooks = (profile_hook if isinstance(profile_hook, (list, tuple))
                     else [profile_hook])
            for hook in hooks:
                with hook():
                    jax.block_until_ready(sharded(*dev_args))

        results = [
            {name: np.asarray(out_arrs[i]).reshape(
                N_CORES, *out_avals[i].shape)[c]
             for i, name in enumerate(out_names)}
            for c in range(N_CORES)
        ]
        return results, exec_ns

    _NC_CACHE[key] = run
    return run


def _make_in_maps(x, w):
    lhsT_all = _build_lhsT_all(w)  # [C, 9, 128, 64]
    in_maps = []
    for k in range(N_CORES):
        c0 = k * CH_PER_CORE
        xc = _pad_core_input(x[:, c0 : c0 + CH_PER_CORE])
        # [ch, 9, 128, 64] -> [ch, K=128, (pass, M=64)]
        import ml_dtypes
        wc = np.ascontiguousarray(
            lhsT_all[c0 : c0 + CH_PER_CORE].transpose(0, 2, 1, 3)
        ).reshape(CH_PER_CORE, 128, 9 * 64).astype(ml_dtypes.bfloat16)
        in_maps.append({"xpad": xc, "wts": wc})
    return in_maps


def kernel(inputs, kernel, _timing_reps=0, _profile_hook=None):
    global LAST_EXEC_NS
    x = np.asarray(inputs, dtype=np.float32)
    w = np.asarray(kernel, dtype=np.float32)
    assert x.shape == (B, C, X, Y, Z, T), x.shape
    assert w.shape == (81, C, 1), w.shape

    run = _get_runner()
    results, exec_ns = run(_make_in_maps(x, w), timing_reps=_timing_reps,
                           profile_hook=_profile_hook)
    LAST_EXEC_NS = exec_ns

    outs = [results[k]["out"].astype(np.float32) for k in range(N_CORES)]
    return np.concatenate(outs, axis=1)
